# revision 21
# baseline (speedup 1.0000x reference)
"""Multi-head self-attention with positional bias, sharded over 8 NeuronCores.

Sharding: head-parallel. Core h computes head h for all batches; the full
output is the sum of the 8 per-core partials (row-parallel Wout), reduced on
host.

Device kernel (per core), fp16 matmul inputs / fp32 PSUM accumulation:
  - projections: packed q|k weight [d, 128] gives one [128, 512]-psum chain
    per token chunk (q rows 0-63, k rows 64-127); v accumulates 16 token
    tiles side by side in one [128, 1024] psum tile.
  - scores are computed TRANSPOSED: ST[j, i] = k_j . q_i so exp's output is
    directly the layout the attention*V matmul needs.
  - the positional bias never touches the PE: host ships E = exp(bias^T) and
    the device computes P~ = exp(ST) * E with a 2x-mode fp16 DVE multiply.
  - softmax denominator: ones column appended to v; PV matmul row 64 then
    holds sum_j P~[j, i]. Normalization happens BEFORE the output projection
    (ot * recip[i], a broadcast fp16 multiply), so the Wout psum tiles DMA
    straight to DRAM with no extra engine pass.
"""

import numpy as np
from contextlib import ExitStack

import concourse.bass as bass
import concourse.bacc as bacc
import concourse.mybir as mybir
import concourse.tile as tile
from concourse.bass_utils import run_bass_kernel_spmd

HEADS = 8
DH = 64
B, N, D = 4, 2048, 512
SCALE = DH ** -0.5
N_CORES = 8

F32 = mybir.dt.float32
F16 = mybir.dt.float16
MUL = mybir.AluOpType.mult


def build_nc(b=B, n=N, d=D, n_cores=1):
    """Per-core Bass program (SPMD: per-head differences come in via inputs)."""
    assert b % 2 == 0 and n % 512 == 0 and d % 128 == 0
    T = b * n
    CC = d // 128        # contraction chunks for projections
    NJ = n // 128        # key tiles (j)
    IC = 512
    NIC = n // IC        # i-chunks of 512
    NIP = NIC // 2       # i-groups of 1024
    NPAIR = b // 2
    VW = DH + 1          # v block width (+1 ones column for denominator)

    nc = bacc.Bacc("TRN2", target_bir_lowering=False, debug=False,
                   num_devices=n_cores)
    qT = nc.declare_dram_parameter("qT", [d, T], F16, isOutput=False)
    eb = nc.declare_dram_parameter("eb", [n, n], F16, isOutput=False)
    wqk = nc.declare_dram_parameter("wqk", [d, 2 * DH], F16, isOutput=False)
    wv = nc.declare_dram_parameter("wv", [d, DH], F16, isOutput=False)
    wout = nc.declare_dram_parameter("wout", [DH, d], F16, isOutput=False)
    out = nc.declare_dram_parameter("out", [T, d], F16, isOutput=True)

    with ExitStack() as ctx:
        tc = ctx.enter_context(tile.TileContext(nc))

        const = ctx.enter_context(tc.tile_pool(name="const", bufs=1))
        qk_pool = ctx.enter_context(tc.tile_pool(name="qkT", bufs=1))
        v_pool = ctx.enter_context(tc.tile_pool(name="v", bufs=1))
        e_pool = ctx.enter_context(tc.tile_pool(name="ebias", bufs=1))
        ot_pool = ctx.enter_context(tc.tile_pool(name="otf", bufs=3))
        qt_pool = ctx.enter_context(tc.tile_pool(name="qt", bufs=6))
        p_pool = ctx.enter_context(tc.tile_pool(name="pexp", bufs=4))
        pr_pool = ctx.enter_context(tc.tile_pool(name="prod", bufs=4))
        osb_pool = ctx.enter_context(tc.tile_pool(name="osb", bufs=4))
        # PSUM: st_pool holds score tiles, projection accumulators and output
        # po tiles (all [128, 1024] f32 = 2 banks); ots holds PV accumulators.
        st_pool = ctx.enter_context(tc.tile_pool(name="st", bufs=2, space="PSUM"))
        ots_pool = ctx.enter_context(tc.tile_pool(name="ots", bufs=4, space="PSUM"))

        zbias = const.tile([128, 1], F32, tag="zbias")
        nc.vector.memset(zbias, 0.0)
        ones16 = const.tile([128, 16], F16, tag="ones16")
        nc.vector.memset(ones16, 1.0)

        wqk_sb = const.tile([128, CC, 2 * DH], F16, tag="wqk")
        nc.sync.dma_start(out=wqk_sb, in_=wqk[:, :].rearrange("(c p) e -> p c e", p=128))
        wv_sb = const.tile([128, CC, DH], F16, tag="wv")
        nc.sync.dma_start(out=wv_sb, in_=wv[:, :].rearrange("(c p) e -> p c e", p=128))
        wout_sb = const.tile([DH, d], F16, tag="wout")
        nc.sync.dma_start(out=wout_sb, in_=wout[:, :])

        qT_sb = [qk_pool.tile([DH, n], F16, tag=f"qT{bb}", name=f"qT{bb}") for bb in range(b)]
        kT_sb = [qk_pool.tile([DH, n], F16, tag=f"kT{bb}", name=f"kT{bb}") for bb in range(b)]
        v_sb = [v_pool.tile([128, NJ * VW], F16, tag=f"v{bb}", name=f"v{bb}") for bb in range(b)]
        for bb in range(b):
            ones_cols = v_sb[bb].rearrange("p (t w) -> p t w", w=VW)[:, :, DH:VW]
            nc.vector.tensor_copy(ones_cols, ones16[:, 0:NJ].rearrange("p (t o) -> p t o", o=1))



        # ---------------- projections (per batch) ----------------
        # DMA order matters (SP queue + DMA engines are serial): qt for the
        # first two batches goes out first so projections start immediately;
        # the E = exp(bias^T) staging streams behind it, and batches 2/3 are
        # loaded + projected while the first score block runs.
        e_sb = []

        def load_e_tiles(j0, j1):
            for jt in range(j0, j1):
                t = e_pool.tile([128, n], F16, tag=f"eb{jt}", name=f"eb{jt}")
                nc.sync.dma_start(out=t, in_=eb[jt * 128:(jt + 1) * 128, :])
                e_sb.append(t)

        def load_qt(bb):
            qt_c = []
            for c in range(CC):
                t = qt_pool.tile([128, n], F16, tag="qt", name=f"qt{bb}_{c}")
                nc.sync.dma_start(out=t, in_=qT[c * 128:(c + 1) * 128, bb * n:(bb + 1) * n])
                qt_c.append(t)
            return qt_c

        def emit_proj_qk(bb, qt_c, hh):
            # q|k packed: psum rows 0-63 = q^T, 64-127 = k^T
            ps = st_pool.tile([128, 2 * IC], F32, tag="st", name=f"pqk{bb}_{hh}")
            for half in range(2):
                cols = slice(half * IC, (half + 1) * IC)
                acols = slice(hh * 2 * IC + half * IC, hh * 2 * IC + (half + 1) * IC)
                for c in range(CC):
                    nc.tensor.matmul(ps[:, cols], lhsT=wqk_sb[:, c, :],
                                     rhs=qt_c[c][:, acols],
                                     start=(c == 0), stop=(c == CC - 1),
                                     skip_group_check=True)
            dcols = slice(hh * 2 * IC, (hh + 1) * 2 * IC)
            nc.vector.tensor_copy(qT_sb[bb][:, dcols], ps[0:DH, :])
            nc.vector.tensor_copy(kT_sb[bb][:, dcols], ps[DH:128, :])

        def emit_proj_v(bb, qt_c):
            # v: 16 token tiles side by side in one [128, 1024] psum tile
            psv = st_pool.tile([128, 2 * IC], F32, tag="st", name=f"pv{bb}")
            for tt in range(NJ):
                for c in range(CC):
                    nc.tensor.matmul(psv[:, tt * DH:(tt + 1) * DH],
                                     lhsT=qt_c[c][:, tt * 128:(tt + 1) * 128],
                                     rhs=wv_sb[:, c, :],
                                     start=(c == 0), stop=(c == CC - 1),
                                     skip_group_check=True)
            vdst = v_sb[bb].rearrange("p (t w) -> p t w", w=VW)[:, :, 0:DH]
            nc.vector.tensor_copy(vdst, psv.rearrange("p (t e) -> p t e", e=DH))

        def emit_proj(bb, qt_c):
            for hh in range(n // (2 * IC)):
                emit_proj_qk(bb, qt_c, hh)
            emit_proj_v(bb, qt_c)

        for bb in range(2):
            emit_proj(bb, load_qt(bb))
        load_e_tiles(0, 6)
        qt_b2 = load_qt(2)
        load_e_tiles(6, 10)
        qt_b3 = load_qt(3)
        load_e_tiles(10, NJ)

        # ---------------- scores + softmax + P~^T V + out-proj ----------------
        # Software-pipelined emission: engines dispatch in-order with a
        # single-slot wait queue, so PV matmuls are emitted DEPTH steps after
        # their qk/exp/prod chain, and the block epilogue (evac, reciprocal,
        # normalize, Wout matmuls, store) is spread into the next block's
        # steps. This keeps the PE/Act queues free of head-of-line stalls.
        exp_fn = mybir.ActivationFunctionType.Exp
        PROD_POOL_JTS = frozenset((1, 3, 5, 7, 11, 13))  # DVE/Pool work split
        DEPTH, POOL_DEPTH = 2, 5

        steps = [(ip, pair, jt, lb)
                 for ip in range(NIP) for pair in range(NPAIR)
                 for jt in range(NJ) for lb in range(2)]
        SPB = NJ * 2  # steps per (ip, pair) block

        ot_ps_blk = {}     # block index -> {(lb, il): psum tile}
        pv_q = []          # (release_step, fn)
        extra_q = []       # (release_step, fn)

        def emit_pv(blk, pair, jt, lb, prod):
            def fn():
                bb = 2 * pair + lb
                for il in range(2):
                    nc.tensor.matmul(
                        ot_ps_blk[blk][(lb, il)],
                        lhsT=v_sb[bb][:, jt * VW:jt * VW + VW],
                        rhs=prod[:, il * IC:(il + 1) * IC],
                        start=(jt == 0), stop=(jt == NJ - 1),
                        skip_group_check=True)
            return fn

        def emit_evac(blk, ip, pair, lb, il):
            def fn():
                of = ot_pool.tile([VW, IC], F16, tag="of", name="of")
                nc.vector.tensor_copy(of, ot_ps_blk[blk][(lb, il)])
                rr = ot_pool.tile([1, IC], F16, tag="rr", name="rr")
                with nc.allow_low_precision("fp16 softmax denom reciprocal"):
                    nc.vector.reciprocal(rr, of[DH:VW, :])
                of_blk[(blk, lb, il)] = (of, rr)
            return fn

        def emit_norm(blk, lb, il):
            def fn():
                of, rr = of_blk[(blk, lb, il)]
                rbc = ot_pool.tile([DH, IC], F16, tag="rb", name="rb")
                nc.gpsimd.partition_broadcast(rbc, rr[0:1, :])
                onrm = ot_pool.tile([DH, IC], F16, tag="on", name="on")
                nc.vector.tensor_tensor(onrm, of[0:DH, :], rbc, MUL)
                onorm_blk[(blk, lb, il)] = onrm
            return fn

        def emit_po(blk, ip, pair, lb, il, tp, last):
            def fn():
                bb = 2 * pair + lb
                ic = ip * 2 + il
                onrm = onorm_blk[(blk, lb, il)]
                po = st_pool.tile([128, 2 * IC], F32, tag="st", name="po")
                for q in range(2):
                    off = (tp * 2 + q) * 128
                    nc.tensor.matmul(
                        po[:, q * IC:(q + 1) * IC],
                        lhsT=onrm[:, off:off + 128], rhs=wout_sb,
                        start=True, stop=True, skip_group_check=True)
                osb = osb_pool.tile([128, 2 * IC], F16, tag="osb")
                if last:  # tail: the Act engine is idle by then
                    nc.scalar.copy(osb, po)
                else:
                    nc.vector.tensor_copy(osb, po)
                r0 = bb * n + ic * IC + tp * 256
                nc.sync.dma_start(
                    out=out[r0:r0 + 256, :].rearrange("(t p) d -> p t d", p=128),
                    in_=osb.rearrange("p (t d) -> p t d", t=2))
            return fn

        onorm_blk = {}
        of_blk = {}
        n_steps = len(steps)
        n_blk = n_steps // SPB
        last_pv_rel = {}   # (blk, lb) -> last release step (keeps psum order)
        proj_w = {8: (2, 0), 11: (2, 1), 14: (2, 2), 17: (3, 0), 20: (3, 1), 23: (3, 2)}
        for s in range(n_steps + POOL_DEPTH + 20):
            if s in proj_w:  # weave batch-2/3 projections into early score steps
                pb, part = proj_w[s]
                qt_c = qt_b2 if pb == 2 else qt_b3
                if part < 2:
                    emit_proj_qk(pb, qt_c, part)
                else:
                    emit_proj_v(pb, qt_c)
            if s < n_steps:
                ip, pair, jt, lb = steps[s]
                blk = s // SPB
                bstart = blk * SPB
                if s % SPB == 0:
                    ot_ps_blk[blk] = {
                        (l2, i2): ots_pool.tile([VW, IC], F32, tag="ot", name="otp")
                        for l2 in range(2) for i2 in range(2)}
                bb = 2 * pair + lb
                st = st_pool.tile([128, 2 * IC], F32, tag="st", name="st")
                for il in range(2):
                    ic = ip * 2 + il
                    nc.tensor.matmul(
                        st[:, il * IC:(il + 1) * IC],
                        lhsT=kT_sb[bb][:, jt * 128:(jt + 1) * 128],
                        rhs=qT_sb[bb][:, ic * IC:(ic + 1) * IC],
                        start=True, stop=True, skip_group_check=True)
                pexp = p_pool.tile([128, 2 * IC], F16, tag="pexp")
                nc.scalar.activation(pexp, st, exp_fn, bias=zbias)
                prod = pr_pool.tile([128, 2 * IC], F16, tag="prod")
                on_pool = jt in PROD_POOL_JTS
                peng = nc.gpsimd if on_pool else nc.vector
                peng.tensor_tensor(
                    prod, pexp, e_sb[jt][:, ip * 2 * IC:(ip + 1) * 2 * IC], MUL)
                rel = s + (POOL_DEPTH if on_pool else DEPTH)
                if jt == 0 and blk > 0:
                    # the block's psum accumulators only free up once the
                    # previous block's evacuation lands
                    rel = max(rel, bstart + 5 + lb)
                # psum group order: start-matmul first, stop-matmul last
                rel = max(rel, last_pv_rel.get((blk, lb), 0))
                last_pv_rel[(blk, lb)] = rel
                pv_q.append((rel, emit_pv(blk, pair, jt, lb, prod)))
                if s % SPB == SPB - 1:  # schedule this block's epilogue
                    base = s + DEPTH + 1  # right after the block's last PV
                    k = 0
                    for l2 in range(2):
                        for i2 in range(2):
                            extra_q.append((base + k, emit_evac(blk, ip, pair, l2, i2)))
                            extra_q.append((base + 4 + k, emit_norm(blk, l2, i2)))
                            for tp in range(2):
                                extra_q.append((base + 8 + 2 * k + tp,
                                                emit_po(blk, ip, pair, l2, i2, tp,
                                                        blk == n_blk - 1)))
                            k += 1
            for q in (pv_q, extra_q):
                ready = [f for r, f in q if r <= s]
                q[:] = [(r, f) for r, f in q if r > s]
                for f in ready:
                    f()
    nc.compile()
    return nc


def make_in_maps(query, pos_bias, Wq, Wk, Wv, Wout, n_cores=N_CORES):
    """Host-side sharding/layout prep. Head h -> core h."""
    query = np.asarray(query, dtype=np.float32)
    pos_bias = np.asarray(pos_bias, dtype=np.float32)
    Wq = np.asarray(Wq, dtype=np.float32)
    Wk = np.asarray(Wk, dtype=np.float32)
    Wv = np.asarray(Wv, dtype=np.float32)
    Wout = np.asarray(Wout, dtype=np.float32)

    b, n, d = query.shape
    qT = np.ascontiguousarray(query.reshape(b * n, d).T.astype(np.float16))
    wq_s = Wq * np.float32(SCALE)
    in_maps = []
    for h in range(n_cores):
        sl = slice(h * DH, (h + 1) * DH)
        in_maps.append({
            "qT": qT,
            "eb": np.ascontiguousarray(np.exp(pos_bias[h].T).astype(np.float16)),
            "wqk": np.ascontiguousarray(
                np.concatenate([wq_s[:, sl], Wk[:, sl]], axis=1).astype(np.float16)),
            "wv": np.ascontiguousarray(Wv[:, sl].astype(np.float16)),
            "wout": np.ascontiguousarray(Wout[sl, :].astype(np.float16)),
        })
    return in_maps


def run_device(in_maps, b=B, n=N, d=D, trace=False, **kw):
    nc = build_nc(b, n, d, n_cores=len(in_maps))
    return run_bass_kernel_spmd(nc, in_maps, list(range(len(in_maps))), trace=trace, **kw)


def assemble(results, b=B, n=N, d=D):
    acc = np.zeros((b * n, d), dtype=np.float32)
    for r in results:
        acc += r["out"]
    return acc.reshape(b, n, d)


def kernel(query, pos_bias, Wq, Wk, Wv, Wout):
    in_maps = make_in_maps(query, pos_bias, Wq, Wk, Wv, Wout)
    res = run_device(in_maps)
    return assemble(res.results)


# revision 22
# speedup vs baseline: 1.0419x; 1.0419x over previous
"""Multi-head self-attention with positional bias, sharded over 8 NeuronCores.

Sharding: head-parallel. Core h computes head h for all batches; the full
output is the sum of the 8 per-core partials (row-parallel Wout), reduced on
host.

Device kernel (per core), fp16 matmul inputs / fp32 PSUM accumulation:
  - projections: packed q|k weight [d, 128] gives one [128, 512]-psum chain
    per token chunk (q rows 0-63, k rows 64-127); v accumulates 16 token
    tiles side by side in one [128, 1024] psum tile.
  - scores are computed TRANSPOSED: ST[j, i] = k_j . q_i so exp's output is
    directly the layout the attention*V matmul needs.
  - the positional bias never touches the PE: host ships E = exp(bias^T) and
    the device computes P~ = exp(ST) * E with a 2x-mode fp16 DVE multiply.
  - softmax denominator: ones column appended to v; PV matmul row 64 then
    holds sum_j P~[j, i]. Normalization happens BEFORE the output projection
    (ot * recip[i], a broadcast fp16 multiply), so the Wout psum tiles DMA
    straight to DRAM with no extra engine pass.
"""

import numpy as np
from contextlib import ExitStack

import concourse.bass as bass
import concourse.bacc as bacc
import concourse.mybir as mybir
import concourse.tile as tile
from concourse.bass_utils import run_bass_kernel_spmd

HEADS = 8
DH = 64
B, N, D = 4, 2048, 512
SCALE = DH ** -0.5
N_CORES = 8

F32 = mybir.dt.float32
F16 = mybir.dt.float16
MUL = mybir.AluOpType.mult


def build_nc(b=B, n=N, d=D, n_cores=1):
    """Per-core Bass program (SPMD: per-head differences come in via inputs)."""
    assert b % 2 == 0 and n % 512 == 0 and d % 128 == 0
    T = b * n
    CC = d // 128        # contraction chunks for projections
    NJ = n // 128        # key tiles (j)
    IC = 512
    NIC = n // IC        # i-chunks of 512
    NIP = NIC // 2       # i-groups of 1024
    NPAIR = b // 2
    VW = DH + 1          # v block width (+1 ones column for denominator)

    nc = bacc.Bacc("TRN2", target_bir_lowering=False, debug=False,
                   num_devices=n_cores)
    qT = nc.declare_dram_parameter("qT", [d, T], F16, isOutput=False)
    eb = nc.declare_dram_parameter("eb", [n, n], F16, isOutput=False)
    wqk = nc.declare_dram_parameter("wqk", [d, 2 * DH], F16, isOutput=False)
    wv = nc.declare_dram_parameter("wv", [d, DH], F16, isOutput=False)
    wout = nc.declare_dram_parameter("wout", [DH, d], F16, isOutput=False)
    out = nc.declare_dram_parameter("out", [T, d], F16, isOutput=True)

    with ExitStack() as ctx:
        tc = ctx.enter_context(tile.TileContext(nc))

        const = ctx.enter_context(tc.tile_pool(name="const", bufs=1))
        qk_pool = ctx.enter_context(tc.tile_pool(name="qkT", bufs=1))
        v_pool = ctx.enter_context(tc.tile_pool(name="v", bufs=1))
        e_pool = ctx.enter_context(tc.tile_pool(name="ebias", bufs=1))
        ot_pool = ctx.enter_context(tc.tile_pool(name="otf", bufs=3))
        qt_pool = ctx.enter_context(tc.tile_pool(name="qt", bufs=6))
        p_pool = ctx.enter_context(tc.tile_pool(name="pexp", bufs=4))
        pr_pool = ctx.enter_context(tc.tile_pool(name="prod", bufs=8))
        osb_pool = ctx.enter_context(tc.tile_pool(name="osb", bufs=4))
        # PSUM: st_pool holds score tiles, projection accumulators and output
        # po tiles (all [128, 1024] f32 = 2 banks); ots holds PV accumulators.
        st_pool = ctx.enter_context(tc.tile_pool(name="st", bufs=2, space="PSUM"))
        ots_pool = ctx.enter_context(tc.tile_pool(name="ots", bufs=4, space="PSUM"))

        zbias = const.tile([128, 1], F32, tag="zbias")
        nc.vector.memset(zbias, 0.0)
        ones16 = const.tile([128, 16], F16, tag="ones16")
        nc.vector.memset(ones16, 1.0)

        wqk_sb = const.tile([128, CC, 2 * DH], F16, tag="wqk")
        nc.sync.dma_start(out=wqk_sb, in_=wqk[:, :].rearrange("(c p) e -> p c e", p=128))
        wv_sb = const.tile([128, CC, DH], F16, tag="wv")
        nc.sync.dma_start(out=wv_sb, in_=wv[:, :].rearrange("(c p) e -> p c e", p=128))
        wout_sb = const.tile([DH, d], F16, tag="wout")
        nc.sync.dma_start(out=wout_sb, in_=wout[:, :])

        qT_sb = [qk_pool.tile([DH, n], F16, tag=f"qT{bb}", name=f"qT{bb}") for bb in range(b)]
        kT_sb = [qk_pool.tile([DH, n], F16, tag=f"kT{bb}", name=f"kT{bb}") for bb in range(b)]
        v_sb = [v_pool.tile([128, NJ * VW], F16, tag=f"v{bb}", name=f"v{bb}") for bb in range(b)]
        for bb in range(b):
            ones_cols = v_sb[bb].rearrange("p (t w) -> p t w", w=VW)[:, :, DH:VW]
            nc.vector.tensor_copy(ones_cols, ones16[:, 0:NJ].rearrange("p (t o) -> p t o", o=1))



        # ---------------- projections (per batch) ----------------
        # DMA order matters (SP queue + DMA engines are serial): qt for the
        # first two batches goes out first so projections start immediately;
        # the E = exp(bias^T) staging streams behind it, and batches 2/3 are
        # loaded + projected while the first score block runs.
        e_sb = []

        def load_e_tiles(j0, j1):
            for jt in range(j0, j1):
                t = e_pool.tile([128, n], F16, tag=f"eb{jt}", name=f"eb{jt}")
                nc.sync.dma_start(out=t, in_=eb[jt * 128:(jt + 1) * 128, :])
                e_sb.append(t)

        def load_qt(bb):
            qt_c = []
            for c in range(CC):
                t = qt_pool.tile([128, n], F16, tag="qt", name=f"qt{bb}_{c}")
                nc.sync.dma_start(out=t, in_=qT[c * 128:(c + 1) * 128, bb * n:(bb + 1) * n])
                qt_c.append(t)
            return qt_c

        def emit_proj_qk(bb, qt_c, hh):
            # q|k packed: psum rows 0-63 = q^T, 64-127 = k^T
            ps = st_pool.tile([128, 2 * IC], F32, tag="st", name=f"pqk{bb}_{hh}")
            for half in range(2):
                cols = slice(half * IC, (half + 1) * IC)
                acols = slice(hh * 2 * IC + half * IC, hh * 2 * IC + (half + 1) * IC)
                for c in range(CC):
                    nc.tensor.matmul(ps[:, cols], lhsT=wqk_sb[:, c, :],
                                     rhs=qt_c[c][:, acols],
                                     start=(c == 0), stop=(c == CC - 1),
                                     skip_group_check=True)
            dcols = slice(hh * 2 * IC, (hh + 1) * 2 * IC)
            nc.vector.tensor_copy(qT_sb[bb][:, dcols], ps[0:DH, :])
            nc.vector.tensor_copy(kT_sb[bb][:, dcols], ps[DH:128, :])

        def emit_proj_v(bb, qt_c):
            # v: 16 token tiles side by side in one [128, 1024] psum tile
            psv = st_pool.tile([128, 2 * IC], F32, tag="st", name=f"pv{bb}")
            for tt in range(NJ):
                for c in range(CC):
                    nc.tensor.matmul(psv[:, tt * DH:(tt + 1) * DH],
                                     lhsT=qt_c[c][:, tt * 128:(tt + 1) * 128],
                                     rhs=wv_sb[:, c, :],
                                     start=(c == 0), stop=(c == CC - 1),
                                     skip_group_check=True)
            vdst = v_sb[bb].rearrange("p (t w) -> p t w", w=VW)[:, :, 0:DH]
            nc.vector.tensor_copy(vdst, psv.rearrange("p (t e) -> p t e", e=DH))

        def emit_proj(bb, qt_c):
            for hh in range(n // (2 * IC)):
                emit_proj_qk(bb, qt_c, hh)
            emit_proj_v(bb, qt_c)

        for bb in range(2):
            emit_proj(bb, load_qt(bb))
        load_e_tiles(0, 6)
        qt_b2 = load_qt(2)
        load_e_tiles(6, 10)
        qt_b3 = load_qt(3)
        load_e_tiles(10, NJ)

        # ---------------- scores + softmax + P~^T V + out-proj ----------------
        # Software-pipelined emission: engines dispatch in-order with a
        # single-slot wait queue, so PV matmuls are emitted DEPTH steps after
        # their qk/exp/prod chain, and the block epilogue (evac, reciprocal,
        # normalize, Wout matmuls, store) is spread into the next block's
        # steps. This keeps the PE/Act queues free of head-of-line stalls.
        exp_fn = mybir.ActivationFunctionType.Exp
        PROD_POOL_JTS = frozenset((2, 5, 8, 11, 14))  # DVE/Pool work split
        DEPTH, POOL_DEPTH = 2, 6

        steps = [(ip, pair, jt, lb)
                 for ip in range(NIP) for pair in range(NPAIR)
                 for jt in range(NJ) for lb in range(2)]
        SPB = NJ * 2  # steps per (ip, pair) block

        ot_ps_blk = {}     # block index -> {(lb, il): psum tile}
        pv_q = []          # (release_step, fn)
        extra_q = []       # (release_step, fn)

        def emit_pv(blk, pair, jt, lb, prod):
            def fn():
                bb = 2 * pair + lb
                for il in range(2):
                    nc.tensor.matmul(
                        ot_ps_blk[blk][(lb, il)],
                        lhsT=v_sb[bb][:, jt * VW:jt * VW + VW],
                        rhs=prod[:, il * IC:(il + 1) * IC],
                        start=(jt == 0), stop=(jt == NJ - 1),
                        skip_group_check=True)
            return fn

        def emit_evac(blk, ip, pair, lb, il):
            def fn():
                of = ot_pool.tile([VW, IC], F16, tag="of", name="of")
                nc.vector.tensor_copy(of, ot_ps_blk[blk][(lb, il)])
                rr = ot_pool.tile([1, IC], F16, tag="rr", name="rr")
                with nc.allow_low_precision("fp16 softmax denom reciprocal"):
                    nc.vector.reciprocal(rr, of[DH:VW, :])
                of_blk[(blk, lb, il)] = (of, rr)
            return fn

        def emit_norm(blk, lb, il):
            def fn():
                of, rr = of_blk[(blk, lb, il)]
                rbc = ot_pool.tile([DH, IC], F16, tag="rb", name="rb")
                nc.gpsimd.partition_broadcast(rbc, rr[0:1, :])
                onrm = ot_pool.tile([DH, IC], F16, tag="on", name="on")
                nc.vector.tensor_tensor(onrm, of[0:DH, :], rbc, MUL)
                onorm_blk[(blk, lb, il)] = onrm
            return fn

        def emit_po(blk, ip, pair, lb, il, tp, last):
            def fn():
                bb = 2 * pair + lb
                ic = ip * 2 + il
                onrm = onorm_blk[(blk, lb, il)]
                po = st_pool.tile([128, 2 * IC], F32, tag="st", name="po")
                for q in range(2):
                    off = (tp * 2 + q) * 128
                    nc.tensor.matmul(
                        po[:, q * IC:(q + 1) * IC],
                        lhsT=onrm[:, off:off + 128], rhs=wout_sb,
                        start=True, stop=True, skip_group_check=True)
                osb = osb_pool.tile([128, 2 * IC], F16, tag="osb")
                if last:  # tail: the Act engine is idle by then
                    nc.scalar.copy(osb, po)
                else:
                    nc.vector.tensor_copy(osb, po)
                r0 = bb * n + ic * IC + tp * 256
                nc.sync.dma_start(
                    out=out[r0:r0 + 256, :].rearrange("(t p) d -> p t d", p=128),
                    in_=osb.rearrange("p (t d) -> p t d", t=2))
            return fn

        onorm_blk = {}
        of_blk = {}
        n_steps = len(steps)
        n_blk = n_steps // SPB
        last_pv_rel = {}   # (blk, lb) -> last release step (keeps psum order)
        proj_w = {8: (2, 0), 11: (2, 1), 14: (2, 2), 17: (3, 0), 20: (3, 1), 23: (3, 2)}
        for s in range(n_steps + POOL_DEPTH + 20):
            if s in proj_w:  # weave batch-2/3 projections into early score steps
                pb, part = proj_w[s]
                qt_c = qt_b2 if pb == 2 else qt_b3
                if part < 2:
                    emit_proj_qk(pb, qt_c, part)
                else:
                    emit_proj_v(pb, qt_c)
            if s < n_steps:
                ip, pair, jt, lb = steps[s]
                blk = s // SPB
                bstart = blk * SPB
                if s % SPB == 0:
                    ot_ps_blk[blk] = {
                        (l2, i2): ots_pool.tile([VW, IC], F32, tag="ot", name="otp")
                        for l2 in range(2) for i2 in range(2)}
                bb = 2 * pair + lb
                st = st_pool.tile([128, 2 * IC], F32, tag="st", name="st")
                for il in range(2):
                    ic = ip * 2 + il
                    nc.tensor.matmul(
                        st[:, il * IC:(il + 1) * IC],
                        lhsT=kT_sb[bb][:, jt * 128:(jt + 1) * 128],
                        rhs=qT_sb[bb][:, ic * IC:(ic + 1) * IC],
                        start=True, stop=True, skip_group_check=True)
                pexp = p_pool.tile([128, 2 * IC], F16, tag="pexp")
                nc.scalar.activation(pexp, st, exp_fn, bias=zbias)
                prod = pr_pool.tile([128, 2 * IC], F16, tag="prod")
                on_pool = jt in PROD_POOL_JTS
                peng = nc.gpsimd if on_pool else nc.vector
                peng.tensor_tensor(
                    prod, pexp, e_sb[jt][:, ip * 2 * IC:(ip + 1) * 2 * IC], MUL)
                rel = s + (POOL_DEPTH if on_pool else DEPTH)
                if jt == 0 and blk > 0:
                    # the block's psum accumulators only free up once the
                    # previous block's evacuation lands
                    rel = max(rel, bstart + 5 + lb)
                # psum group order: start-matmul first, stop-matmul last
                rel = max(rel, last_pv_rel.get((blk, lb), 0))
                last_pv_rel[(blk, lb)] = rel
                pv_q.append((rel, emit_pv(blk, pair, jt, lb, prod)))
                if s % SPB == SPB - 1:  # schedule this block's epilogue
                    base = s + DEPTH + 1  # right after the block's last PV
                    k = 0
                    for l2 in range(2):
                        for i2 in range(2):
                            extra_q.append((base + k, emit_evac(blk, ip, pair, l2, i2)))
                            extra_q.append((base + 4 + k, emit_norm(blk, l2, i2)))
                            for tp in range(2):
                                extra_q.append((base + 8 + 2 * k + tp,
                                                emit_po(blk, ip, pair, l2, i2, tp,
                                                        blk == n_blk - 1)))
                            k += 1
            for q in (pv_q, extra_q):
                ready = [f for r, f in q if r <= s]
                q[:] = [(r, f) for r, f in q if r > s]
                for f in ready:
                    f()
    nc.compile()
    return nc


def make_in_maps(query, pos_bias, Wq, Wk, Wv, Wout, n_cores=N_CORES):
    """Host-side sharding/layout prep. Head h -> core h."""
    query = np.asarray(query, dtype=np.float32)
    pos_bias = np.asarray(pos_bias, dtype=np.float32)
    Wq = np.asarray(Wq, dtype=np.float32)
    Wk = np.asarray(Wk, dtype=np.float32)
    Wv = np.asarray(Wv, dtype=np.float32)
    Wout = np.asarray(Wout, dtype=np.float32)

    b, n, d = query.shape
    qT = np.ascontiguousarray(query.reshape(b * n, d).T.astype(np.float16))
    wq_s = Wq * np.float32(SCALE)
    in_maps = []
    for h in range(n_cores):
        sl = slice(h * DH, (h + 1) * DH)
        in_maps.append({
            "qT": qT,
            "eb": np.ascontiguousarray(np.exp(pos_bias[h].T).astype(np.float16)),
            "wqk": np.ascontiguousarray(
                np.concatenate([wq_s[:, sl], Wk[:, sl]], axis=1).astype(np.float16)),
            "wv": np.ascontiguousarray(Wv[:, sl].astype(np.float16)),
            "wout": np.ascontiguousarray(Wout[sl, :].astype(np.float16)),
        })
    return in_maps


def run_device(in_maps, b=B, n=N, d=D, trace=False, **kw):
    nc = build_nc(b, n, d, n_cores=len(in_maps))
    return run_bass_kernel_spmd(nc, in_maps, list(range(len(in_maps))), trace=trace, **kw)


def assemble(results, b=B, n=N, d=D):
    acc = np.zeros((b * n, d), dtype=np.float32)
    for r in results:
        acc += r["out"]
    return acc.reshape(b, n, d)


def kernel(query, pos_bias, Wq, Wk, Wv, Wout):
    in_maps = make_in_maps(query, pos_bias, Wq, Wk, Wv, Wout)
    res = run_device(in_maps)
    return assemble(res.results)


# revision 23
# speedup vs baseline: 1.1603x; 1.1137x over previous
"""Multi-head self-attention with positional bias, sharded over 8 NeuronCores.

Sharding: head-parallel. Core h computes head h for all batches; the full
output is the sum of the 8 per-core partials (row-parallel Wout), reduced on
host.

Device kernel (per core), fp16 matmul inputs / fp32 PSUM accumulation:
  - projections: packed q|k weight [d, 128] gives one [128, 512]-psum chain
    per token chunk (q rows 0-63, k rows 64-127); v accumulates 16 token
    tiles side by side in one [128, 1024] psum tile.
  - scores are computed TRANSPOSED: ST[j, i] = k_j . q_i so exp's output is
    directly the layout the attention*V matmul needs.
  - the positional bias never touches the PE: host ships E = exp(bias^T) and
    the device computes P~ = exp(ST) * E with a 2x-mode fp16 DVE multiply.
  - softmax denominator: ones column appended to v; PV matmul row 64 then
    holds sum_j P~[j, i]. Normalization happens BEFORE the output projection
    (ot * recip[i], a broadcast fp16 multiply), so the Wout psum tiles DMA
    straight to DRAM with no extra engine pass.
"""

import numpy as np
from contextlib import ExitStack

import concourse.bass as bass
import concourse.bacc as bacc
import concourse.mybir as mybir
import concourse.tile as tile
from concourse.bass_utils import run_bass_kernel_spmd

HEADS = 8
DH = 64
B, N, D = 4, 2048, 512
SCALE = DH ** -0.5
N_CORES = 8

F32 = mybir.dt.float32
F16 = mybir.dt.float16
MUL = mybir.AluOpType.mult


def build_nc(b=B, n=N, d=D, n_cores=1):
    """Per-core Bass program (SPMD: per-head differences come in via inputs)."""
    assert b % 2 == 0 and n % 512 == 0 and d % 128 == 0
    T = b * n
    CC = d // 128        # contraction chunks for projections
    NJ = n // 128        # key tiles (j)
    IC = 512
    NIC = n // IC        # i-chunks of 512
    NIP = NIC // 2       # i-groups of 1024
    NPAIR = b // 2
    VW = DH + 1          # v block width (+1 ones column for denominator)

    nc = bacc.Bacc("TRN2", target_bir_lowering=False, debug=False,
                   num_devices=n_cores)
    qT = nc.declare_dram_parameter("qT", [d, T], F16, isOutput=False)
    eb = nc.declare_dram_parameter("eb", [n, n], F16, isOutput=False)
    wqk = nc.declare_dram_parameter("wqk", [d, 2 * DH], F16, isOutput=False)
    wv = nc.declare_dram_parameter("wv", [d, DH], F16, isOutput=False)
    wout = nc.declare_dram_parameter("wout", [DH, d], F16, isOutput=False)
    out = nc.declare_dram_parameter("out", [T, d], F16, isOutput=True)

    with ExitStack() as ctx:
        tc = ctx.enter_context(tile.TileContext(nc))

        const = ctx.enter_context(tc.tile_pool(name="const", bufs=1))
        qk_pool = ctx.enter_context(tc.tile_pool(name="qkT", bufs=1))
        v_pool = ctx.enter_context(tc.tile_pool(name="v", bufs=1))
        e_pool = ctx.enter_context(tc.tile_pool(name="ebias", bufs=1))
        ot_pool = ctx.enter_context(tc.tile_pool(name="otf", bufs=3))
        qt_pool = ctx.enter_context(tc.tile_pool(name="qt", bufs=6))
        p_pool = ctx.enter_context(tc.tile_pool(name="pexp", bufs=4))
        pr_pool = ctx.enter_context(tc.tile_pool(name="prod", bufs=8))
        osb_pool = ctx.enter_context(tc.tile_pool(name="osb", bufs=4))
        # PSUM: st_pool holds score tiles, projection accumulators and output
        # po tiles (all [128, 1024] f32 = 2 banks); ots holds PV accumulators.
        st_pool = ctx.enter_context(tc.tile_pool(name="st", bufs=2, space="PSUM"))
        ots_pool = ctx.enter_context(tc.tile_pool(name="ots", bufs=4, space="PSUM"))

        zbias = const.tile([128, 1], F32, tag="zbias")
        nc.vector.memset(zbias, 0.0)
        ones16 = const.tile([128, 16], F16, tag="ones16")
        nc.vector.memset(ones16, 1.0)

        wqk_sb = const.tile([128, CC, 2 * DH], F16, tag="wqk")
        nc.sync.dma_start(out=wqk_sb, in_=wqk[:, :].rearrange("(c p) e -> p c e", p=128))
        wv_sb = const.tile([128, CC, DH], F16, tag="wv")
        nc.sync.dma_start(out=wv_sb, in_=wv[:, :].rearrange("(c p) e -> p c e", p=128))
        wout_sb = const.tile([DH, d], F16, tag="wout")
        nc.sync.dma_start(out=wout_sb, in_=wout[:, :])

        qT_sb = [qk_pool.tile([DH, n], F16, tag=f"qT{bb}", name=f"qT{bb}") for bb in range(b)]
        kT_sb = [qk_pool.tile([DH, n], F16, tag=f"kT{bb}", name=f"kT{bb}") for bb in range(b)]
        v_sb = [v_pool.tile([128, NJ * VW], F16, tag=f"v{bb}", name=f"v{bb}") for bb in range(b)]
        for bb in range(b):
            ones_cols = v_sb[bb].rearrange("p (t w) -> p t w", w=VW)[:, :, DH:VW]
            nc.vector.tensor_copy(ones_cols, ones16[:, 0:NJ].rearrange("p (t o) -> p t o", o=1))



        # ---------------- projections (per batch) ----------------
        # DMA order matters (SP queue + DMA engines are serial): qt for the
        # first two batches goes out first so projections start immediately;
        # the E = exp(bias^T) staging streams behind it, and batches 2/3 are
        # loaded + projected while the first score block runs.
        e_sb = []

        def load_e_tiles(j0, j1):
            for jt in range(j0, j1):
                t = e_pool.tile([128, n], F16, tag=f"eb{jt}", name=f"eb{jt}")
                nc.sync.dma_start(out=t, in_=eb[jt * 128:(jt + 1) * 128, :])
                e_sb.append(t)

        def load_qt(bb):
            qt_c = []
            for c in range(CC):
                t = qt_pool.tile([128, n], F16, tag="qt", name=f"qt{bb}_{c}")
                nc.sync.dma_start(out=t, in_=qT[c * 128:(c + 1) * 128, bb * n:(bb + 1) * n])
                qt_c.append(t)
            return qt_c

        def emit_proj_qk(bb, qt_c, hh, act_evac=False):
            # q|k packed: psum rows 0-63 = q^T, 64-127 = k^T
            ps = st_pool.tile([128, 2 * IC], F32, tag="st", name=f"pqk{bb}_{hh}")
            for half in range(2):
                cols = slice(half * IC, (half + 1) * IC)
                acols = slice(hh * 2 * IC + half * IC, hh * 2 * IC + (half + 1) * IC)
                for c in range(CC):
                    nc.tensor.matmul(ps[:, cols], lhsT=wqk_sb[:, c, :],
                                     rhs=qt_c[c][:, acols],
                                     start=(c == 0), stop=(c == CC - 1),
                                     skip_group_check=True)
            dcols = slice(hh * 2 * IC, (hh + 1) * 2 * IC)
            if act_evac:  # the Act engine is idle until the first score tile
                nc.scalar.copy(qT_sb[bb][:, dcols], ps[0:DH, :])
                nc.scalar.copy(kT_sb[bb][:, dcols], ps[DH:128, :])
            else:
                nc.vector.tensor_copy(qT_sb[bb][:, dcols], ps[0:DH, :])
                nc.vector.tensor_copy(kT_sb[bb][:, dcols], ps[DH:128, :])

        def emit_proj_v(bb, qt_c, act_evac=False):
            # v: 16 token tiles side by side in one [128, 1024] psum tile
            psv = st_pool.tile([128, 2 * IC], F32, tag="st", name=f"pv{bb}")
            for tt in range(NJ):
                for c in range(CC):
                    nc.tensor.matmul(psv[:, tt * DH:(tt + 1) * DH],
                                     lhsT=qt_c[c][:, tt * 128:(tt + 1) * 128],
                                     rhs=wv_sb[:, c, :],
                                     start=(c == 0), stop=(c == CC - 1),
                                     skip_group_check=True)
            vdst = v_sb[bb].rearrange("p (t w) -> p t w", w=VW)[:, :, 0:DH]
            src = psv.rearrange("p (t e) -> p t e", e=DH)
            if act_evac:
                nc.scalar.copy(vdst, src)
            else:
                nc.vector.tensor_copy(vdst, src)

        for bb in range(2):
            qt_c = load_qt(bb)
            for hh in range(n // (2 * IC)):
                emit_proj_qk(bb, qt_c, hh, act_evac=True)
            emit_proj_v(bb, qt_c, act_evac=True)
        load_e_tiles(0, 6)
        qt_b2 = load_qt(2)
        load_e_tiles(6, 10)
        qt_b3 = load_qt(3)
        load_e_tiles(10, NJ)

        # ---------------- scores + softmax + P~^T V + out-proj ----------------
        # Software-pipelined emission: engines dispatch in-order with a
        # single-slot wait queue, so PV matmuls are emitted DEPTH steps after
        # their qk/exp/prod chain, and the block epilogue (evac, reciprocal,
        # normalize, Wout matmuls, store) is spread into the next block's
        # steps. This keeps the PE/Act queues free of head-of-line stalls.
        exp_fn = mybir.ActivationFunctionType.Exp
        PROD_POOL_JTS = frozenset()  # pool multiply is too slow for the PV path
        DEPTH, POOL_DEPTH = 2, 6

        steps = [(ip, pair, jt, lb)
                 for ip in range(NIP) for pair in range(NPAIR)
                 for jt in range(NJ) for lb in range(2)]
        SPB = NJ * 2  # steps per (ip, pair) block

        ot_ps_blk = {}     # block index -> {(lb, il): psum tile}
        pv_q = []          # (release_step, fn)
        extra_q = []       # (release_step, fn)

        def emit_pv(blk, pair, jt, lb, prod):
            def fn():
                bb = 2 * pair + lb
                for il in range(2):
                    nc.tensor.matmul(
                        ot_ps_blk[blk][(lb, il)],
                        lhsT=v_sb[bb][:, jt * VW:jt * VW + VW],
                        rhs=prod[:, il * IC:(il + 1) * IC],
                        start=(jt == 0), stop=(jt == NJ - 1),
                        skip_group_check=True)
            return fn

        def emit_evac(blk, ip, pair, lb, il):
            def fn():
                of = ot_pool.tile([VW, IC], F16, tag="of", name="of")
                nc.vector.tensor_copy(of, ot_ps_blk[blk][(lb, il)])
                rr = ot_pool.tile([1, IC], F16, tag="rr", name="rr")
                with nc.allow_low_precision("fp16 softmax denom reciprocal"):
                    nc.vector.reciprocal(rr, of[DH:VW, :])
                of_blk[(blk, lb, il)] = (of, rr)
            return fn

        def emit_norm(blk, lb, il):
            def fn():
                of, rr = of_blk[(blk, lb, il)]
                rbc = ot_pool.tile([DH, IC], F16, tag="rb", name="rb")
                nc.gpsimd.partition_broadcast(rbc, rr[0:1, :])
                onrm = ot_pool.tile([DH, IC], F16, tag="on", name="on")
                nc.vector.tensor_tensor(onrm, of[0:DH, :], rbc, MUL)
                onorm_blk[(blk, lb, il)] = onrm
            return fn

        def emit_po(blk, ip, pair, lb, il, tp, last):
            def fn():
                bb = 2 * pair + lb
                ic = ip * 2 + il
                onrm = onorm_blk[(blk, lb, il)]
                po = st_pool.tile([128, 2 * IC], F32, tag="st", name="po")
                for q in range(2):
                    off = (tp * 2 + q) * 128
                    nc.tensor.matmul(
                        po[:, q * IC:(q + 1) * IC],
                        lhsT=onrm[:, off:off + 128], rhs=wout_sb,
                        start=True, stop=True, skip_group_check=True)
                osb = osb_pool.tile([128, 2 * IC], F16, tag="osb")
                if last:  # tail: the Act engine is idle by then
                    nc.scalar.copy(osb, po)
                else:
                    nc.vector.tensor_copy(osb, po)
                r0 = bb * n + ic * IC + tp * 256
                nc.sync.dma_start(
                    out=out[r0:r0 + 256, :].rearrange("(t p) d -> p t d", p=128),
                    in_=osb.rearrange("p (t d) -> p t d", t=2))
            return fn

        onorm_blk = {}
        of_blk = {}
        n_steps = len(steps)
        n_blk = n_steps // SPB
        last_pv_rel = {}   # (blk, lb) -> last release step (keeps psum order)
        proj_w = {8: (2, 0), 11: (2, 1), 14: (2, 2), 17: (3, 0), 20: (3, 1), 23: (3, 2)}
        for s in range(n_steps + POOL_DEPTH + 20):
            if s in proj_w:  # weave batch-2/3 projections into early score steps
                pb, part = proj_w[s]
                qt_c = qt_b2 if pb == 2 else qt_b3
                if part < 2:
                    emit_proj_qk(pb, qt_c, part)
                else:
                    emit_proj_v(pb, qt_c)
            if s < n_steps:
                ip, pair, jt, lb = steps[s]
                blk = s // SPB
                bstart = blk * SPB
                if s % SPB == 0:
                    ot_ps_blk[blk] = {
                        (l2, i2): ots_pool.tile([VW, IC], F32, tag="ot", name="otp")
                        for l2 in range(2) for i2 in range(2)}
                bb = 2 * pair + lb
                st = st_pool.tile([128, 2 * IC], F32, tag="st", name="st")
                for il in range(2):
                    ic = ip * 2 + il
                    nc.tensor.matmul(
                        st[:, il * IC:(il + 1) * IC],
                        lhsT=kT_sb[bb][:, jt * 128:(jt + 1) * 128],
                        rhs=qT_sb[bb][:, ic * IC:(ic + 1) * IC],
                        start=True, stop=True, skip_group_check=True)
                pexp = p_pool.tile([128, 2 * IC], F16, tag="pexp")
                nc.scalar.activation(pexp, st, exp_fn, bias=zbias)
                prod = pr_pool.tile([128, 2 * IC], F16, tag="prod")
                on_pool = jt in PROD_POOL_JTS
                peng = nc.gpsimd if on_pool else nc.vector
                peng.tensor_tensor(
                    prod, pexp, e_sb[jt][:, ip * 2 * IC:(ip + 1) * 2 * IC], MUL)
                rel = s + (POOL_DEPTH if on_pool else DEPTH)
                if jt == 0 and blk > 0:
                    # the block's psum accumulators only free up once the
                    # previous block's evacuation lands
                    rel = max(rel, bstart + 5 + lb)
                # psum group order: start-matmul first, stop-matmul last
                rel = max(rel, last_pv_rel.get((blk, lb), 0))
                last_pv_rel[(blk, lb)] = rel
                pv_q.append((rel, emit_pv(blk, pair, jt, lb, prod)))
                if s % SPB == SPB - 1:  # schedule this block's epilogue
                    base = s + DEPTH + 1  # right after the block's last PV
                    k = 0
                    for l2 in range(2):
                        for i2 in range(2):
                            extra_q.append((base + k, emit_evac(blk, ip, pair, l2, i2)))
                            extra_q.append((base + 4 + k, emit_norm(blk, l2, i2)))
                            for tp in range(2):
                                extra_q.append((base + 8 + 2 * k + tp,
                                                emit_po(blk, ip, pair, l2, i2, tp,
                                                        blk == n_blk - 1)))
                            k += 1
            for q in (pv_q, extra_q):
                ready = [f for r, f in q if r <= s]
                q[:] = [(r, f) for r, f in q if r > s]
                for f in ready:
                    f()
    nc.compile()
    return nc


def make_in_maps(query, pos_bias, Wq, Wk, Wv, Wout, n_cores=N_CORES):
    """Host-side sharding/layout prep. Head h -> core h."""
    query = np.asarray(query, dtype=np.float32)
    pos_bias = np.asarray(pos_bias, dtype=np.float32)
    Wq = np.asarray(Wq, dtype=np.float32)
    Wk = np.asarray(Wk, dtype=np.float32)
    Wv = np.asarray(Wv, dtype=np.float32)
    Wout = np.asarray(Wout, dtype=np.float32)

    b, n, d = query.shape
    qT = np.ascontiguousarray(query.reshape(b * n, d).T.astype(np.float16))
    wq_s = Wq * np.float32(SCALE)
    in_maps = []
    for h in range(n_cores):
        sl = slice(h * DH, (h + 1) * DH)
        in_maps.append({
            "qT": qT,
            "eb": np.ascontiguousarray(np.exp(pos_bias[h].T).astype(np.float16)),
            "wqk": np.ascontiguousarray(
                np.concatenate([wq_s[:, sl], Wk[:, sl]], axis=1).astype(np.float16)),
            "wv": np.ascontiguousarray(Wv[:, sl].astype(np.float16)),
            "wout": np.ascontiguousarray(Wout[sl, :].astype(np.float16)),
        })
    return in_maps


def run_device(in_maps, b=B, n=N, d=D, trace=False, **kw):
    nc = build_nc(b, n, d, n_cores=len(in_maps))
    return run_bass_kernel_spmd(nc, in_maps, list(range(len(in_maps))), trace=trace, **kw)


def assemble(results, b=B, n=N, d=D):
    acc = np.zeros((b * n, d), dtype=np.float32)
    for r in results:
        acc += r["out"]
    return acc.reshape(b, n, d)


def kernel(query, pos_bias, Wq, Wk, Wv, Wout):
    in_maps = make_in_maps(query, pos_bias, Wq, Wk, Wv, Wout)
    res = run_device(in_maps)
    return assemble(res.results)


# revision 24
# speedup vs baseline: 1.2222x; 1.0533x over previous
"""Multi-head self-attention with positional bias, sharded over 8 NeuronCores.

Sharding: head-parallel. Core h computes head h for all batches; the full
output is the sum of the 8 per-core partials (row-parallel Wout), reduced on
host.

Device kernel (per core), fp16 matmul inputs / fp32 PSUM accumulation:
  - projections: packed q|k weight [d, 128] gives one [128, 512]-psum chain
    per token chunk (q rows 0-63, k rows 64-127); v accumulates 16 token
    tiles side by side in one [128, 1024] psum tile.
  - scores are computed TRANSPOSED: ST[j, i] = k_j . q_i so exp's output is
    directly the layout the attention*V matmul needs.
  - the positional bias never touches the PE: host ships E = exp(bias^T) and
    the device computes P~ = exp(ST) * E with a 2x-mode fp16 DVE multiply.
  - softmax denominator: ones column appended to v; PV matmul row 64 then
    holds sum_j P~[j, i]. Normalization happens BEFORE the output projection
    (ot * recip[i], a broadcast fp16 multiply), so the Wout psum tiles DMA
    straight to DRAM with no extra engine pass.
"""

import numpy as np
from contextlib import ExitStack

import concourse.bass as bass
import concourse.bacc as bacc
import concourse.mybir as mybir
import concourse.tile as tile
from concourse.bass_utils import run_bass_kernel_spmd

HEADS = 8
DH = 64
B, N, D = 4, 2048, 512
SCALE = DH ** -0.5
N_CORES = 8

F32 = mybir.dt.float32
F16 = mybir.dt.float16
MUL = mybir.AluOpType.mult


def build_nc(b=B, n=N, d=D, n_cores=1):
    """Per-core Bass program (SPMD: per-head differences come in via inputs)."""
    assert b % 2 == 0 and n % 512 == 0 and d % 128 == 0
    T = b * n
    CC = d // 128        # contraction chunks for projections
    NJ = n // 128        # key tiles (j)
    IC = 512
    NIC = n // IC        # i-chunks of 512
    NIP = NIC // 2       # i-groups of 1024
    NPAIR = b // 2
    VW = DH + 1          # v block width (+1 ones column for denominator)

    nc = bacc.Bacc("TRN2", target_bir_lowering=False, debug=False,
                   num_devices=n_cores)
    qT = nc.declare_dram_parameter("qT", [d, T], F16, isOutput=False)
    eb = nc.declare_dram_parameter("eb", [n, n], F16, isOutput=False)
    wqk = nc.declare_dram_parameter("wqk", [d, 2 * DH], F16, isOutput=False)
    wv = nc.declare_dram_parameter("wv", [d, DH], F16, isOutput=False)
    wout = nc.declare_dram_parameter("wout", [DH, d], F16, isOutput=False)
    out = nc.declare_dram_parameter("out", [T, d], F16, isOutput=True)

    with ExitStack() as ctx:
        tc = ctx.enter_context(tile.TileContext(nc))

        const = ctx.enter_context(tc.tile_pool(name="const", bufs=1))
        qk_pool = ctx.enter_context(tc.tile_pool(name="qkT", bufs=1))
        v_pool = ctx.enter_context(tc.tile_pool(name="v", bufs=1))
        e_pool = ctx.enter_context(tc.tile_pool(name="ebias", bufs=1))
        ot_pool = ctx.enter_context(tc.tile_pool(name="otf", bufs=3))
        qt_pool = ctx.enter_context(tc.tile_pool(name="qt", bufs=6))
        p_pool = ctx.enter_context(tc.tile_pool(name="pexp", bufs=4))
        pr_pool = ctx.enter_context(tc.tile_pool(name="prod", bufs=8))
        osb_pool = ctx.enter_context(tc.tile_pool(name="osb", bufs=4))
        # PSUM: st_pool holds score tiles, projection accumulators and output
        # po tiles (all [128, 1024] f32 = 2 banks); ots holds PV accumulators.
        st_pool = ctx.enter_context(tc.tile_pool(name="st", bufs=2, space="PSUM"))
        ots_pool = ctx.enter_context(tc.tile_pool(name="ots", bufs=4, space="PSUM"))

        zbias = const.tile([128, 1], F32, tag="zbias")
        nc.vector.memset(zbias, 0.0)
        ones16 = const.tile([128, 16], F16, tag="ones16")
        nc.vector.memset(ones16, 1.0)

        wqk_sb = const.tile([128, CC, 2 * DH], F16, tag="wqk")
        nc.sync.dma_start(out=wqk_sb, in_=wqk[:, :].rearrange("(c p) e -> p c e", p=128))
        wv_sb = const.tile([128, CC, DH], F16, tag="wv")
        nc.sync.dma_start(out=wv_sb, in_=wv[:, :].rearrange("(c p) e -> p c e", p=128))
        wout_sb = const.tile([DH, d], F16, tag="wout")
        nc.sync.dma_start(out=wout_sb, in_=wout[:, :])

        qT_sb = [qk_pool.tile([DH, n], F16, tag=f"qT{bb}", name=f"qT{bb}") for bb in range(b)]
        kT_sb = [qk_pool.tile([DH, n], F16, tag=f"kT{bb}", name=f"kT{bb}") for bb in range(b)]
        v_sb = [v_pool.tile([128, NJ * VW], F16, tag=f"v{bb}", name=f"v{bb}") for bb in range(b)]
        for bb in range(b):
            ones_cols = v_sb[bb].rearrange("p (t w) -> p t w", w=VW)[:, :, DH:VW]
            nc.vector.tensor_copy(ones_cols, ones16[:, 0:NJ].rearrange("p (t o) -> p t o", o=1))



        # ---------------- projections (per batch) ----------------
        # DMA order matters (SP queue + DMA engines are serial): qt for the
        # first two batches goes out first so projections start immediately;
        # the E = exp(bias^T) staging streams behind it, and batches 2/3 are
        # loaded + projected while the first score block runs.
        e_sb = []

        def load_e_tiles(j0, j1):
            for jt in range(j0, j1):
                t = e_pool.tile([128, n], F16, tag=f"eb{jt}", name=f"eb{jt}")
                nc.sync.dma_start(out=t, in_=eb[jt * 128:(jt + 1) * 128, :])
                e_sb.append(t)

        def load_qt(bb):
            qt_c = []
            for c in range(CC):
                t = qt_pool.tile([128, n], F16, tag="qt", name=f"qt{bb}_{c}")
                nc.sync.dma_start(out=t, in_=qT[c * 128:(c + 1) * 128, bb * n:(bb + 1) * n])
                qt_c.append(t)
            return qt_c

        def emit_proj_qk(bb, qt_c, hh, act_evac=False):
            # q|k packed: psum rows 0-63 = q^T, 64-127 = k^T
            ps = st_pool.tile([128, 2 * IC], F32, tag="st", name=f"pqk{bb}_{hh}")
            for half in range(2):
                cols = slice(half * IC, (half + 1) * IC)
                acols = slice(hh * 2 * IC + half * IC, hh * 2 * IC + (half + 1) * IC)
                for c in range(CC):
                    nc.tensor.matmul(ps[:, cols], lhsT=wqk_sb[:, c, :],
                                     rhs=qt_c[c][:, acols],
                                     start=(c == 0), stop=(c == CC - 1),
                                     skip_group_check=True)
            dcols = slice(hh * 2 * IC, (hh + 1) * 2 * IC)
            if act_evac:  # the Act engine is idle until the first score tile
                nc.scalar.copy(qT_sb[bb][:, dcols], ps[0:DH, :])
                nc.scalar.copy(kT_sb[bb][:, dcols], ps[DH:128, :])
            else:
                nc.vector.tensor_copy(qT_sb[bb][:, dcols], ps[0:DH, :])
                nc.vector.tensor_copy(kT_sb[bb][:, dcols], ps[DH:128, :])

        def emit_proj_v(bb, qt_c, act_evac=False):
            # v: 16 token tiles side by side in one [128, 1024] psum tile
            psv = st_pool.tile([128, 2 * IC], F32, tag="st", name=f"pv{bb}")
            for tt in range(NJ):
                for c in range(CC):
                    nc.tensor.matmul(psv[:, tt * DH:(tt + 1) * DH],
                                     lhsT=qt_c[c][:, tt * 128:(tt + 1) * 128],
                                     rhs=wv_sb[:, c, :],
                                     start=(c == 0), stop=(c == CC - 1),
                                     skip_group_check=True)
            vdst = v_sb[bb].rearrange("p (t w) -> p t w", w=VW)[:, :, 0:DH]
            src = psv.rearrange("p (t e) -> p t e", e=DH)
            if act_evac:
                nc.scalar.copy(vdst, src)
            else:
                nc.vector.tensor_copy(vdst, src)

        for bb in range(2):
            qt_c = load_qt(bb)
            for hh in range(n // (2 * IC)):
                emit_proj_qk(bb, qt_c, hh, act_evac=True)
            emit_proj_v(bb, qt_c, act_evac=True)
        load_e_tiles(0, 6)
        qt_b2 = load_qt(2)
        load_e_tiles(6, 10)
        qt_b3 = load_qt(3)
        load_e_tiles(10, NJ)

        # ---------------- scores + softmax + P~^T V + out-proj ----------------
        # Software-pipelined emission: engines dispatch in-order with a
        # single-slot wait queue, so PV matmuls are emitted DEPTH steps after
        # their qk/exp/prod chain, and the block epilogue (evac, reciprocal,
        # normalize, Wout matmuls, store) is spread into the next block's
        # steps. This keeps the PE/Act queues free of head-of-line stalls.
        exp_fn = mybir.ActivationFunctionType.Exp
        PROD_POOL_JTS = frozenset()  # pool multiply is too slow for the PV path
        DEPTH, POOL_DEPTH = 2, 6

        steps = [(ip, pair, jt, lb)
                 for ip in range(NIP) for pair in range(NPAIR)
                 for jt in range(NJ) for lb in range(2)]
        SPB = NJ * 2  # steps per (ip, pair) block

        ot_ps_blk = {}     # block index -> {(lb, il): psum tile}
        pv_q = []          # (release_step, fn)
        extra_q = []       # (release_step, fn)

        def emit_pv(blk, pair, jt, lb, prod):
            def fn():
                bb = 2 * pair + lb
                for il in range(2):
                    nc.tensor.matmul(
                        ot_ps_blk[blk][(lb, il)],
                        lhsT=v_sb[bb][:, jt * VW:jt * VW + VW],
                        rhs=prod[:, il * IC:(il + 1) * IC],
                        start=(jt == 0), stop=(jt == NJ - 1),
                        skip_group_check=True)
            return fn

        def emit_evac(blk, ip, pair, lb, il):
            def fn():
                of = ot_pool.tile([VW, IC], F16, tag="of", name="of")
                nc.vector.tensor_copy(of, ot_ps_blk[blk][(lb, il)])
                of_blk[(blk, lb, il)] = of
            return fn

        def emit_norm(blk, lb, il):
            def fn():
                of = of_blk[(blk, lb, il)]
                dbc = ot_pool.tile([DH, IC], F16, tag="rb", name="rb")
                nc.gpsimd.partition_broadcast(dbc, of[DH:VW, :])
                onrm = ot_pool.tile([DH, IC], F16, tag="on", name="on")
                with nc.allow_low_precision("fp16 softmax normalize"):
                    nc.gpsimd.tensor_tensor(onrm, of[0:DH, :], dbc, mybir.AluOpType.divide)
                onorm_blk[(blk, lb, il)] = onrm
            return fn

        def emit_po(blk, ip, pair, lb, il, tp, last):
            def fn():
                bb = 2 * pair + lb
                ic = ip * 2 + il
                onrm = onorm_blk[(blk, lb, il)]
                po = st_pool.tile([128, 2 * IC], F32, tag="st", name="po")
                for q in range(2):
                    off = (tp * 2 + q) * 128
                    nc.tensor.matmul(
                        po[:, q * IC:(q + 1) * IC],
                        lhsT=onrm[:, off:off + 128], rhs=wout_sb,
                        start=True, stop=True, skip_group_check=True)
                osb = osb_pool.tile([128, 2 * IC], F16, tag="osb")
                if last:  # tail: the Act engine is idle by then
                    nc.scalar.copy(osb, po)
                else:
                    nc.vector.tensor_copy(osb, po)
                r0 = bb * n + ic * IC + tp * 256
                nc.sync.dma_start(
                    out=out[r0:r0 + 256, :].rearrange("(t p) d -> p t d", p=128),
                    in_=osb.rearrange("p (t d) -> p t d", t=2))
            return fn

        onorm_blk = {}
        of_blk = {}
        n_steps = len(steps)
        n_blk = n_steps // SPB
        last_pv_rel = {}   # (blk, lb) -> last release step (keeps psum order)
        proj_w = {8: (2, 0), 11: (2, 1), 14: (2, 2), 17: (3, 0), 20: (3, 1), 23: (3, 2)}
        for s in range(n_steps + POOL_DEPTH + 20):
            if s in proj_w:  # weave batch-2/3 projections into early score steps
                pb, part = proj_w[s]
                qt_c = qt_b2 if pb == 2 else qt_b3
                if part < 2:
                    emit_proj_qk(pb, qt_c, part)
                else:
                    emit_proj_v(pb, qt_c)
            if s < n_steps:
                ip, pair, jt, lb = steps[s]
                blk = s // SPB
                bstart = blk * SPB
                if s % SPB == 0:
                    ot_ps_blk[blk] = {
                        (l2, i2): ots_pool.tile([VW, IC], F32, tag="ot", name="otp")
                        for l2 in range(2) for i2 in range(2)}
                bb = 2 * pair + lb
                st = st_pool.tile([128, 2 * IC], F32, tag="st", name="st")
                for il in range(2):
                    ic = ip * 2 + il
                    nc.tensor.matmul(
                        st[:, il * IC:(il + 1) * IC],
                        lhsT=kT_sb[bb][:, jt * 128:(jt + 1) * 128],
                        rhs=qT_sb[bb][:, ic * IC:(ic + 1) * IC],
                        start=True, stop=True, skip_group_check=True)
                pexp = p_pool.tile([128, 2 * IC], F16, tag="pexp")
                nc.scalar.activation(pexp, st, exp_fn, bias=zbias)
                prod = pr_pool.tile([128, 2 * IC], F16, tag="prod")
                on_pool = jt in PROD_POOL_JTS
                peng = nc.gpsimd if on_pool else nc.vector
                peng.tensor_tensor(
                    prod, pexp, e_sb[jt][:, ip * 2 * IC:(ip + 1) * 2 * IC], MUL)
                rel = s + (POOL_DEPTH if on_pool else DEPTH)
                if jt == 0 and blk > 0:
                    # the block's psum accumulators only free up once the
                    # previous block's evacuation lands
                    rel = max(rel, bstart + 5 + lb)
                # psum group order: start-matmul first, stop-matmul last
                rel = max(rel, last_pv_rel.get((blk, lb), 0))
                last_pv_rel[(blk, lb)] = rel
                pv_q.append((rel, emit_pv(blk, pair, jt, lb, prod)))
                if s % SPB == SPB - 1:  # schedule this block's epilogue
                    base = s + DEPTH + 1  # right after the block's last PV
                    k = 0
                    for l2 in range(2):
                        for i2 in range(2):
                            extra_q.append((base + k, emit_evac(blk, ip, pair, l2, i2)))
                            extra_q.append((base + 4 + k, emit_norm(blk, l2, i2)))
                            for tp in range(2):
                                extra_q.append((base + 8 + 2 * k + tp,
                                                emit_po(blk, ip, pair, l2, i2, tp,
                                                        blk == n_blk - 1)))
                            k += 1
            for q in (pv_q, extra_q):
                ready = [f for r, f in q if r <= s]
                q[:] = [(r, f) for r, f in q if r > s]
                for f in ready:
                    f()
    nc.compile()
    return nc


def make_in_maps(query, pos_bias, Wq, Wk, Wv, Wout, n_cores=N_CORES):
    """Host-side sharding/layout prep. Head h -> core h."""
    query = np.asarray(query, dtype=np.float32)
    pos_bias = np.asarray(pos_bias, dtype=np.float32)
    Wq = np.asarray(Wq, dtype=np.float32)
    Wk = np.asarray(Wk, dtype=np.float32)
    Wv = np.asarray(Wv, dtype=np.float32)
    Wout = np.asarray(Wout, dtype=np.float32)

    b, n, d = query.shape
    qT = np.ascontiguousarray(query.reshape(b * n, d).T.astype(np.float16))
    wq_s = Wq * np.float32(SCALE)
    in_maps = []
    for h in range(n_cores):
        sl = slice(h * DH, (h + 1) * DH)
        in_maps.append({
            "qT": qT,
            "eb": np.ascontiguousarray(np.exp(pos_bias[h].T).astype(np.float16)),
            "wqk": np.ascontiguousarray(
                np.concatenate([wq_s[:, sl], Wk[:, sl]], axis=1).astype(np.float16)),
            "wv": np.ascontiguousarray(Wv[:, sl].astype(np.float16)),
            "wout": np.ascontiguousarray(Wout[sl, :].astype(np.float16)),
        })
    return in_maps


def run_device(in_maps, b=B, n=N, d=D, trace=False, **kw):
    nc = build_nc(b, n, d, n_cores=len(in_maps))
    return run_bass_kernel_spmd(nc, in_maps, list(range(len(in_maps))), trace=trace, **kw)


def assemble(results, b=B, n=N, d=D):
    acc = np.zeros((b * n, d), dtype=np.float32)
    for r in results:
        acc += r["out"]
    return acc.reshape(b, n, d)


def kernel(query, pos_bias, Wq, Wk, Wv, Wout):
    in_maps = make_in_maps(query, pos_bias, Wq, Wk, Wv, Wout)
    res = run_device(in_maps)
    return assemble(res.results)
